# revision 35
# baseline (speedup 1.0000x reference)
"""Trainium2 8-core kernel for nn_AttentionMechanism_51049981281163.

Reference module: multi-head attention, B=2, S=2048, D=1024, H=16 heads,
head_dim=64, fp32, mask all-ones, biases all-zero.

Sharding: batch x head-group tensor parallel. Core c handles batch b=c//4
and head group g=c%4 (4 heads = 256 of the 1024 hidden dims). Wq/Wk/Wv are
split column-wise, Wo row-wise; each core computes a partial [S, D] output
and the host sums the 4 partials per batch (the "unshard" of row-parallel
Wo) and adds bo.

Device kernel (per core), bf16 matmul inputs with fp32 PSUM accumulate:
  - host passes x[b].T pre-tiled, so projections need no on-device transpose
  - QT/KT computed in [head_dim, S] layout; V in [S, head_dim] layout with a
    ones-column appended per head (softmax denominators ride along free in
    the context matmul's extra output row)
  - scores computed transposed [k, q]; the two heads of a pair use T0/T8
    64-row PE tiles into one two-bank PSUM tile, so the pair streams
    concurrently; one ScalarE exp per k-tile covers both heads with the
    1/sqrt(hd) scale folded in
  - context accumulated over k-tiles in PSUM; the only PSUM reader is a
    ScalarE copy (fast slot turnaround); the reciprocal + K=1-matmul
    partition-broadcast + multiply trail one block behind on VectorE/PE
  - output projection from the context (already in lhsT layout) x Wo shard
"""

import sys

sys.path.insert(0, "/opt/trn_rl_repo")

import numpy as np

B, S, D, H = 2, 2048, 1024, 16
HD = D // H          # 64
SCALE = HD ** -0.5
NCORES = 8
GROUPS = 4           # head groups (cores per batch)
HPG = H // GROUPS    # 4 heads per group/core
DL = HPG * HD        # 256 local hidden dims per core
VW = HD + 1          # V block width incl. ones column


def _build_graph():
    import concourse.tile as tile
    from concourse import bacc, mybir

    F32 = mybir.dt.float32
    BF16 = mybir.dt.bfloat16
    Exp = mybir.ActivationFunctionType.Exp
    mult = mybir.AluOpType.mult
    add = mybir.AluOpType.add

    nc = bacc.Bacc("TRN2")

    # x[b].T tiled: [p, c, s] = x[b][s, c*128+p]
    xt_e = nc.declare_dram_parameter("xt", [128, 8, S], BF16, isOutput=False)
    # W[:, gsl] tiled: [p, c, d] = W[c*128+p, g*256+d]
    wq_e = nc.declare_dram_parameter("wq", [128, 8, DL], BF16, isOutput=False)
    wk_e = nc.declare_dram_parameter("wk", [128, 8, DL], BF16, isOutput=False)
    wv_e = nc.declare_dram_parameter("wv", [128, 8, DL], BF16, isOutput=False)
    # Wo[gsl, :] tiled: [p, kc, dd] = Wo[g*256+kc*128+p, dd]
    wo_e = nc.declare_dram_parameter("wo", [128, 2, D], BF16, isOutput=False)
    out_e = nc.declare_dram_parameter("out", [S, D], BF16, isOutput=True)

    with nc.allow_low_precision(reason="bf16 compute, 2e-2 tolerance"), \
         tile.TileContext(nc) as tc:
        with tc.tile_pool(name="big", bufs=1) as big:
            xt_sb = big.tile([128, 8, S], BF16)
            wq_sb = big.tile([128, 8, DL], BF16)
            wk_sb = big.tile([128, 8, DL], BF16)
            wv_sb = big.tile([128, 8, DL], BF16)
            wo_sb = big.tile([128, 2, D], BF16)
            qt_sb = big.tile([128, 2, S], BF16)
            kt_sb = big.tile([128, 2, S], BF16)
            vp_sb = big.tile([128, 16, HPG, VW], BF16)
            ctx_sb = big.tile([128, 2, S], BF16)
            ones_sb = big.tile([1, 64], BF16)

            nc.vector.memset(ones_sb[:], 1.0)
            warm_sb = big.tile([128, 640], BF16)
            nc.vector.memset(warm_sb[:], 0.5)
            nc.sync.dma_start(out=wv_sb[:], in_=wv_e[:])
            for c in range(8):
                nc.sync.dma_start(out=xt_sb[:, c, :], in_=xt_e[:, c, :])
            nc.sync.dma_start(out=wq_sb[:], in_=wq_e[:])
            nc.sync.dma_start(out=wk_sb[:], in_=wk_e[:])
            nc.sync.dma_start(out=wo_sb[:], in_=wo_e[:])

            # ---- Phase A: projections (V first: see wait-count note) ----
            with tc.tile_pool(name="pp", bufs=2, space="PSUM") as ppool, \
                 tc.tile_pool(name="pv", bufs=2, space="PSUM") as pvpool:
                # dummy full-array matmuls during the input-DMA wait keep the
                # PE HAM un-throttled from the start (no data dependencies)
                for _ in range(2):
                    pw = ppool.tile([128, 512], F32, tag="pp", name="pw")
                    for r in range(12):
                        nc.tensor.matmul(
                            pw[:],
                            lhsT=warm_sb[:, 0:128],
                            rhs=warm_sb[:, 128:640],
                            start=(r == 0), stop=(r == 11),
                        )
                nc.vector.memset(vp_sb[:, :, :, HD], 1.0)
                for kt in range(16):
                    pv = pvpool.tile([128, DL], F32)
                    for c in range(8):
                        nc.tensor.matmul(
                            pv[:],
                            lhsT=xt_sb[:, c, kt * 128:(kt + 1) * 128],
                            rhs=wv_sb[:, c, :],
                            start=(c == 0), stop=(c == 7),
                        )
                    nc.vector.tensor_copy(
                        vp_sb[:, kt, :, 0:HD],
                        pv[:].rearrange("p (h d) -> p h d", h=HPG))
                for w_sb, o_sb in ((wk_sb, kt_sb), (wq_sb, qt_sb)):
                    for m in range(2):
                        for n4 in range(4):
                            pp = ppool.tile([128, 512], F32)
                            for c in range(8):
                                nc.tensor.matmul(
                                    pp[:],
                                    lhsT=w_sb[:, c, m * 128:(m + 1) * 128],
                                    rhs=xt_sb[:, c, n4 * 512:(n4 + 1) * 512],
                                    start=(c == 0), stop=(c == 7),
                                )
                            nc.vector.tensor_copy(
                                o_sb[:, m, n4 * 512:(n4 + 1) * 512], pp[:])

            # ---- Phase B: attention, Phase C: output projection ----
            with tc.tile_pool(name="ps", bufs=3, space="PSUM") as pspool, \
                 tc.tile_pool(name="pc", bufs=2, space="PSUM") as pcpool, \
                 tc.tile_pool(name="pt", bufs=4) as ptpool, \
                 tc.tile_pool(name="cu", bufs=4) as cupool, \
                 tc.tile_pool(name="sm", bufs=3) as smpool, \
                 tc.tile_pool(name="ob", bufs=2) as obpool:
                def block_loop(n2, qc, j):
                    # heads 2j/2j+1, q-chunk of 512. Both heads' scores land
                    # in ONE [128,1024] psum tile (separate banks), so the
                    # T0/T8 row-tiled pair issues together when the slot
                    # frees; one exp covers both heads.
                    q0 = n2 * 1024 + qc * 512
                    qh = slice(q0, q0 + 512)
                    pcs = []
                    for _ in range(2):
                        pcs.append(pcpool.tile([HD + 1, 512], F32, tag="pc",
                                               name="pc"))
                    for kt in range(16):
                        ksl = slice(kt * 128, (kt + 1) * 128)
                        ps = pspool.tile([128, 1024], F32, tag="ps", name="ps")
                        for o in range(2):
                            nc.tensor.matmul(
                                ps[:, o * 512:(o + 1) * 512],
                                lhsT=kt_sb[64 * o:64 * o + 64, j, ksl],
                                rhs=qt_sb[64 * o:64 * o + 64, j, qh],
                                start=True, stop=True,
                                tile_position=(64 * o, 0),
                            )
                        pt = ptpool.tile([128, 1024], BF16, tag="pt",
                                         name="pt")
                        nc.scalar.activation(pt[:], ps[:], Exp, scale=SCALE)
                        for o in range(2):
                            nc.tensor.matmul(
                                pcs[o][:],
                                lhsT=vp_sb[:, kt, 2 * j + o, :],
                                rhs=pt[:, o * 512:(o + 1) * 512],
                                start=(kt == 0), stop=(kt == 15),
                            )
                    cus = []
                    for o in range(2):
                        # sole reader of pc is this DVE copy; it sits at the
                        # head of DVE's queue (before the deferred reciprocal
                        # chain) so the pc slot frees quickly, and ScalarE
                        # stays dedicated to exp
                        cu = cupool.tile([HD + 1, 512], F32, tag="cu",
                                         name="cu")
                        nc.vector.tensor_copy(cu[:], pcs[o][:])
                        cus.append(cu)
                    return cus

                def finish_norm(n2, qc, h, cu):
                    hp = 64 * (h % 2)
                    m = h // 2
                    q0 = n2 * 1024 + qc * 512
                    qsl = slice(q0, q0 + 512)
                    rinv_bf = smpool.tile([1, 512], BF16, tag="rbf")
                    nc.vector.reciprocal(rinv_bf[:], cu[HD:HD + 1, :])
                    # partition-broadcast 1/denom via K=1 matmul
                    pb = pspool.tile([64, 512], F32, tag="ps")
                    nc.tensor.matmul(
                        pb[:], lhsT=ones_sb[:], rhs=rinv_bf[:],
                        start=True, stop=True,
                        tile_position=(0, 0))
                    rb = smpool.tile([64, 512], F32, tag="rb")
                    nc.vector.tensor_copy(rb[:], pb[:])
                    nc.vector.tensor_tensor(
                        out=ctx_sb[hp:hp + 64, m, qsl],
                        in0=cu[0:HD, :], in1=rb[:], op=mult)

                def wo_stage(n2, qc):
                    for qt in range(4):
                        qg = n2 * 8 + qc * 4 + qt
                        ob = obpool.tile([128, D], BF16)
                        for nn in range(2):
                            po = pspool.tile([128, 512], F32, tag="ps",
                                             name="po")
                            for kc in range(2):
                                nc.tensor.matmul(
                                    po[:],
                                    lhsT=ctx_sb[:, kc, qg * 128:(qg + 1) * 128],
                                    rhs=wo_sb[:, kc, nn * 512:(nn + 1) * 512],
                                    start=(kc == 0), stop=(kc == 1),
                                )
                            nc.vector.tensor_copy(
                                ob[:, nn * 512:(nn + 1) * 512], po[:])
                        nc.sync.dma_start(
                            out=out_e[qg * 128:(qg + 1) * 128, :], in_=ob[:])

                blocks = [(n2, qc, j) for n2 in range(2)
                          for qc in range(2) for j in range(2)]
                pend = None
                for n2, qc, j in blocks:
                    cus = block_loop(n2, qc, j)
                    if pend is not None:
                        pn2, pqc, pj, pcus = pend
                        finish_norm(pn2, pqc, 2 * pj, pcus[0])
                        finish_norm(pn2, pqc, 2 * pj + 1, pcus[1])
                        if pj == 1:
                            wo_stage(pn2, pqc)
                    pend = (n2, qc, j, cus)
                pn2, pqc, pj, pcus = pend
                finish_norm(pn2, pqc, 2 * pj, pcus[0])
                finish_norm(pn2, pqc, 2 * pj + 1, pcus[1])
                wo_stage(pn2, pqc)
    nc.compile()
    return nc


def _shard_inputs(x, Wq, Wk, Wv, Wo):
    """Build the 8 per-core input maps (host-side layout prep, bf16)."""
    import ml_dtypes

    bf16 = ml_dtypes.bfloat16
    in_maps = []
    xtb = [
        np.ascontiguousarray(
            x[b].T.reshape(8, 128, S).transpose(1, 0, 2)).astype(bf16)
        for b in range(B)
    ]
    for core in range(NCORES):
        b, g = divmod(core, GROUPS)
        gsl = slice(g * DL, (g + 1) * DL)
        wq = np.ascontiguousarray(
            Wq[:, gsl].reshape(8, 128, DL).transpose(1, 0, 2)).astype(bf16)
        wk = np.ascontiguousarray(
            Wk[:, gsl].reshape(8, 128, DL).transpose(1, 0, 2)).astype(bf16)
        wv = np.ascontiguousarray(
            Wv[:, gsl].reshape(8, 128, DL).transpose(1, 0, 2)).astype(bf16)
        wo = np.ascontiguousarray(
            Wo[gsl, :].reshape(2, 128, D).transpose(1, 0, 2)).astype(bf16)
        in_maps.append(
            {"xt": xtb[b], "wq": wq, "wk": wk, "wv": wv, "wo": wo})
    return in_maps


def _gather(results, bo):
    out = np.zeros((B, S, D), dtype=np.float32)
    for core in range(NCORES):
        b = core // GROUPS
        out[b] += results[core]["out"].astype(np.float32)
    out += bo.astype(np.float32)
    return out


def _run_device(x, Wq, Wk, Wv, Wo, bo, trace=False, tmpdir=None):
    from concourse.bass_utils import run_bass_kernel_spmd

    nc = _build_graph()
    in_maps = _shard_inputs(x, Wq, Wk, Wv, Wo)
    bkr = run_bass_kernel_spmd(
        nc, in_maps, core_ids=list(range(NCORES)), trace=trace, tmpdir=tmpdir)
    return _gather(bkr.results, bo), bkr


def _reference_numpy(x, mask, Wq, bq, Wk, bk, Wv, bv, Wo, bo):
    """Exact fallback for inputs outside the hardcoded spec."""
    b, s, d = x.shape
    h = H if d % H == 0 else 1
    hd = d // h
    q = (x @ Wq + bq).reshape(b, s, h, hd).transpose(0, 2, 1, 3)
    k = (x @ Wk + bk).reshape(b, s, h, hd).transpose(0, 2, 1, 3)
    v = (x @ Wv + bv).reshape(b, s, h, hd).transpose(0, 2, 1, 3)
    scores = np.einsum("bhqd,bhkd->bhqk", q, k) * (hd ** -0.5)
    scores = np.where(mask[:, None, None, :] == 0, -np.inf, scores)
    scores -= scores.max(axis=-1, keepdims=True)
    e = np.exp(scores)
    attn = e / e.sum(axis=-1, keepdims=True)
    ctx = np.einsum("bhqk,bhkd->bhqd", attn, v)
    ctx = ctx.transpose(0, 2, 1, 3).reshape(b, s, d)
    return (ctx @ Wo + bo).astype(np.float32)


def kernel(x, mask, Wq, bq, Wk, bk, Wv, bv, Wo, bo):
    x = np.asarray(x, dtype=np.float32)
    mask = np.asarray(mask)
    Wq, bq = np.asarray(Wq, np.float32), np.asarray(bq, np.float32)
    Wk, bk = np.asarray(Wk, np.float32), np.asarray(bk, np.float32)
    Wv, bv = np.asarray(Wv, np.float32), np.asarray(bv, np.float32)
    Wo, bo = np.asarray(Wo, np.float32), np.asarray(bo, np.float32)

    general = (
        x.shape != (B, S, D)
        or not np.all(mask == 1)
        or any(np.any(t != 0) for t in (bq, bk, bv))
    )
    if general:
        return _reference_numpy(x, mask, Wq, bq, Wk, bk, Wv, bv, Wo, bo)

    out, _ = _run_device(x, Wq, Wk, Wv, Wo, bo)
    return out


# revision 36
# speedup vs baseline: 1.1887x; 1.1887x over previous
"""Trainium2 8-core kernel for nn_AttentionMechanism_51049981281163.

Reference module: multi-head attention, B=2, S=2048, D=1024, H=16 heads,
head_dim=64, fp32, mask all-ones, biases all-zero.

Sharding: batch x head-group tensor parallel. Core c handles batch b=c//4
and head group g=c%4 (4 heads = 256 of the 1024 hidden dims). Wq/Wk/Wv are
split column-wise, Wo row-wise; each core computes a partial [S, D] output
and the host sums the 4 partials per batch (the "unshard" of row-parallel
Wo) and adds bo.

Device kernel (per core), bf16 matmul inputs with fp32 PSUM accumulate:
  - host passes x[b].T pre-tiled, so projections need no on-device transpose
  - QT/KT computed in [head_dim, S] layout; V in [S, head_dim] layout with a
    ones-column appended per head (softmax denominators ride along free in
    the context matmul's extra output row)
  - scores computed transposed [k, q]; the two heads of a pair use T0/T8
    64-row PE tiles into one two-bank PSUM tile, so the pair streams
    concurrently; one ScalarE exp per k-tile covers both heads with the
    1/sqrt(hd) scale folded in
  - context accumulated over k-tiles in PSUM; the only PSUM reader is a
    ScalarE copy (fast slot turnaround); the reciprocal + K=1-matmul
    partition-broadcast + multiply trail one block behind on VectorE/PE
  - output projection from the context (already in lhsT layout) x Wo shard
"""

import sys

sys.path.insert(0, "/opt/trn_rl_repo")

import numpy as np

B, S, D, H = 2, 2048, 1024, 16
HD = D // H          # 64
SCALE = HD ** -0.5
NCORES = 8
GROUPS = 4           # head groups (cores per batch)
HPG = H // GROUPS    # 4 heads per group/core
DL = HPG * HD        # 256 local hidden dims per core
VW = HD + 1          # V block width incl. ones column


def _build_graph():
    import concourse.tile as tile
    from concourse import bacc, mybir

    F32 = mybir.dt.float32
    BF16 = mybir.dt.bfloat16
    Exp = mybir.ActivationFunctionType.Exp
    mult = mybir.AluOpType.mult
    add = mybir.AluOpType.add

    nc = bacc.Bacc("TRN2")

    # x[b].T tiled: [p, c, s] = x[b][s, c*128+p]
    xt_e = nc.declare_dram_parameter("xt", [128, 8, S], BF16, isOutput=False)
    # W[:, gsl] tiled: [p, c, d] = W[c*128+p, g*256+d]
    wq_e = nc.declare_dram_parameter("wq", [128, 8, DL], BF16, isOutput=False)
    wk_e = nc.declare_dram_parameter("wk", [128, 8, DL], BF16, isOutput=False)
    wv_e = nc.declare_dram_parameter("wv", [128, 8, DL], BF16, isOutput=False)
    # Wo[gsl, :] tiled: [p, kc, dd] = Wo[g*256+kc*128+p, dd]
    wo_e = nc.declare_dram_parameter("wo", [128, 2, D], BF16, isOutput=False)
    out_e = nc.declare_dram_parameter("out", [S, D], BF16, isOutput=True)

    with nc.allow_low_precision(reason="bf16 compute, 2e-2 tolerance"), \
         tile.TileContext(nc) as tc:
        with tc.tile_pool(name="big", bufs=1) as big:
            xt_sb = big.tile([128, 8, S], BF16)
            wq_sb = big.tile([128, 8, DL], BF16)
            wk_sb = big.tile([128, 8, DL], BF16)
            wv_sb = big.tile([128, 8, DL], BF16)
            wo_sb = big.tile([128, 2, D], BF16)
            qt_sb = big.tile([128, 2, S], BF16)
            kt_sb = big.tile([128, 2, S], BF16)
            vp_sb = big.tile([128, 16, HPG, VW], BF16)
            ctx_sb = big.tile([128, 2, S], BF16)
            ones_sb = big.tile([1, 64], BF16)

            nc.vector.memset(ones_sb[:], 1.0)
            warm_sb = big.tile([128, 640], BF16)
            nc.vector.memset(warm_sb[:], 0.5)
            nc.sync.dma_start(out=wv_sb[:], in_=wv_e[:])
            for c in range(8):
                nc.sync.dma_start(out=xt_sb[:, c, :], in_=xt_e[:, c, :])
            nc.sync.dma_start(out=wq_sb[:], in_=wq_e[:])
            nc.sync.dma_start(out=wk_sb[:], in_=wk_e[:])
            nc.sync.dma_start(out=wo_sb[:], in_=wo_e[:])

            # ---- Phase A: projections (V first: see wait-count note) ----
            with tc.tile_pool(name="pp", bufs=2, space="PSUM") as ppool, \
                 tc.tile_pool(name="pv", bufs=2, space="PSUM") as pvpool:
                # dummy full-array matmuls during the input-DMA wait keep the
                # PE HAM un-throttled from the start (no data dependencies)
                for _ in range(2):
                    pw = ppool.tile([128, 512], F32, tag="pp", name="pw")
                    for r in range(12):
                        nc.tensor.matmul(
                            pw[:],
                            lhsT=warm_sb[:, 0:128],
                            rhs=warm_sb[:, 128:640],
                            start=(r == 0), stop=(r == 11),
                        )
                nc.vector.memset(vp_sb[:, :, :, HD], 1.0)
                for kt in range(16):
                    pv = pvpool.tile([128, DL], F32)
                    for c in range(8):
                        nc.tensor.matmul(
                            pv[:],
                            lhsT=xt_sb[:, c, kt * 128:(kt + 1) * 128],
                            rhs=wv_sb[:, c, :],
                            start=(c == 0), stop=(c == 7),
                        )
                    nc.vector.tensor_copy(
                        vp_sb[:, kt, :, 0:HD],
                        pv[:].rearrange("p (h d) -> p h d", h=HPG))
                for w_sb, o_sb in ((wk_sb, kt_sb), (wq_sb, qt_sb)):
                    for m in range(2):
                        for n4 in range(4):
                            pp = ppool.tile([128, 512], F32)
                            for c in range(8):
                                nc.tensor.matmul(
                                    pp[:],
                                    lhsT=w_sb[:, c, m * 128:(m + 1) * 128],
                                    rhs=xt_sb[:, c, n4 * 512:(n4 + 1) * 512],
                                    start=(c == 0), stop=(c == 7),
                                )
                            nc.vector.tensor_copy(
                                o_sb[:, m, n4 * 512:(n4 + 1) * 512], pp[:])

            # ---- Phase B: attention, Phase C: output projection ----
            with tc.tile_pool(name="ps", bufs=3, space="PSUM") as pspool, \
                 tc.tile_pool(name="pc", bufs=2, space="PSUM") as pcpool, \
                 tc.tile_pool(name="pt", bufs=4) as ptpool, \
                 tc.tile_pool(name="cu", bufs=4) as cupool, \
                 tc.tile_pool(name="sm", bufs=3) as smpool, \
                 tc.tile_pool(name="ob", bufs=2) as obpool:
                def block_loop(n2, qc, j):
                    # heads 2j/2j+1, q-chunk of 512. Both heads' scores land
                    # in ONE [128,1024] psum tile (separate banks), so the
                    # T0/T8 row-tiled pair issues together when the slot
                    # frees; one exp covers both heads.
                    q0 = n2 * 1024 + qc * 512
                    qh = slice(q0, q0 + 512)
                    pcs = []
                    for _ in range(2):
                        pcs.append(pcpool.tile([HD + 1, 512], F32, tag="pc",
                                               name="pc"))
                    for kt in range(16):
                        ksl = slice(kt * 128, (kt + 1) * 128)
                        ps = pspool.tile([128, 1024], F32, tag="ps", name="ps")
                        for o in range(2):
                            nc.tensor.matmul(
                                ps[:, o * 512:(o + 1) * 512],
                                lhsT=kt_sb[64 * o:64 * o + 64, j, ksl],
                                rhs=qt_sb[64 * o:64 * o + 64, j, qh],
                                start=True, stop=True,
                                tile_position=(64 * o, 0),
                            )
                        pt = ptpool.tile([128, 1024], BF16, tag="pt",
                                         name="pt")
                        nc.scalar.activation(pt[:], ps[:], Exp, scale=SCALE)
                        for o in range(2):
                            nc.tensor.matmul(
                                pcs[o][:],
                                lhsT=vp_sb[:, kt, 2 * j + o, :],
                                rhs=pt[:, o * 512:(o + 1) * 512],
                                start=(kt == 0), stop=(kt == 15),
                            )
                    cus = []
                    for o in range(2):
                        # sole reader of pc is this ACT copy -> pc-slot
                        # reuse WAR lands on the ACT sem (fast), while the
                        # slow DVE reciprocal chain trails behind
                        cu = cupool.tile([HD + 1, 512], F32, tag="cu",
                                         name="cu")
                        nc.scalar.copy(cu[:], pcs[o][:])
                        cus.append(cu)
                    return cus

                def finish_norm(n2, qc, h, cu):
                    hp = 64 * (h % 2)
                    m = h // 2
                    q0 = n2 * 1024 + qc * 512
                    qsl = slice(q0, q0 + 512)
                    rinv_bf = smpool.tile([1, 512], BF16, tag="rbf")
                    nc.vector.reciprocal(rinv_bf[:], cu[HD:HD + 1, :])
                    # partition-broadcast 1/denom via K=1 matmul
                    pb = pspool.tile([64, 512], F32, tag="ps")
                    nc.tensor.matmul(
                        pb[:], lhsT=ones_sb[:], rhs=rinv_bf[:],
                        start=True, stop=True,
                        tile_position=(0, 0))
                    rb = smpool.tile([64, 512], F32, tag="rb")
                    nc.vector.tensor_copy(rb[:], pb[:])
                    nc.vector.tensor_tensor(
                        out=ctx_sb[hp:hp + 64, m, qsl],
                        in0=cu[0:HD, :], in1=rb[:], op=mult)

                def wo_stage(n2, qc):
                    for qt in range(4):
                        qg = n2 * 8 + qc * 4 + qt
                        ob = obpool.tile([128, D], BF16)
                        for nn in range(2):
                            po = pspool.tile([128, 512], F32, tag="ps",
                                             name="po")
                            for kc in range(2):
                                nc.tensor.matmul(
                                    po[:],
                                    lhsT=ctx_sb[:, kc, qg * 128:(qg + 1) * 128],
                                    rhs=wo_sb[:, kc, nn * 512:(nn + 1) * 512],
                                    start=(kc == 0), stop=(kc == 1),
                                )
                            nc.vector.tensor_copy(
                                ob[:, nn * 512:(nn + 1) * 512], po[:])
                        nc.sync.dma_start(
                            out=out_e[qg * 128:(qg + 1) * 128, :], in_=ob[:])

                blocks = [(n2, qc, j) for n2 in range(2)
                          for qc in range(2) for j in range(2)]
                pend = None
                for n2, qc, j in blocks:
                    cus = block_loop(n2, qc, j)
                    if pend is not None:
                        pn2, pqc, pj, pcus = pend
                        finish_norm(pn2, pqc, 2 * pj, pcus[0])
                        finish_norm(pn2, pqc, 2 * pj + 1, pcus[1])
                        if pj == 1:
                            wo_stage(pn2, pqc)
                    pend = (n2, qc, j, cus)
                pn2, pqc, pj, pcus = pend
                finish_norm(pn2, pqc, 2 * pj, pcus[0])
                finish_norm(pn2, pqc, 2 * pj + 1, pcus[1])
                wo_stage(pn2, pqc)
    nc.compile()
    return nc


def _shard_inputs(x, Wq, Wk, Wv, Wo):
    """Build the 8 per-core input maps (host-side layout prep, bf16)."""
    import ml_dtypes

    bf16 = ml_dtypes.bfloat16
    in_maps = []
    xtb = [
        np.ascontiguousarray(
            x[b].T.reshape(8, 128, S).transpose(1, 0, 2)).astype(bf16)
        for b in range(B)
    ]
    for core in range(NCORES):
        b, g = divmod(core, GROUPS)
        gsl = slice(g * DL, (g + 1) * DL)
        wq = np.ascontiguousarray(
            Wq[:, gsl].reshape(8, 128, DL).transpose(1, 0, 2)).astype(bf16)
        wk = np.ascontiguousarray(
            Wk[:, gsl].reshape(8, 128, DL).transpose(1, 0, 2)).astype(bf16)
        wv = np.ascontiguousarray(
            Wv[:, gsl].reshape(8, 128, DL).transpose(1, 0, 2)).astype(bf16)
        wo = np.ascontiguousarray(
            Wo[gsl, :].reshape(2, 128, D).transpose(1, 0, 2)).astype(bf16)
        in_maps.append(
            {"xt": xtb[b], "wq": wq, "wk": wk, "wv": wv, "wo": wo})
    return in_maps


def _gather(results, bo):
    out = np.zeros((B, S, D), dtype=np.float32)
    for core in range(NCORES):
        b = core // GROUPS
        out[b] += results[core]["out"].astype(np.float32)
    out += bo.astype(np.float32)
    return out


def _run_device(x, Wq, Wk, Wv, Wo, bo, trace=False, tmpdir=None):
    from concourse.bass_utils import run_bass_kernel_spmd

    nc = _build_graph()
    in_maps = _shard_inputs(x, Wq, Wk, Wv, Wo)
    bkr = run_bass_kernel_spmd(
        nc, in_maps, core_ids=list(range(NCORES)), trace=trace, tmpdir=tmpdir)
    return _gather(bkr.results, bo), bkr


def _reference_numpy(x, mask, Wq, bq, Wk, bk, Wv, bv, Wo, bo):
    """Exact fallback for inputs outside the hardcoded spec."""
    b, s, d = x.shape
    h = H if d % H == 0 else 1
    hd = d // h
    q = (x @ Wq + bq).reshape(b, s, h, hd).transpose(0, 2, 1, 3)
    k = (x @ Wk + bk).reshape(b, s, h, hd).transpose(0, 2, 1, 3)
    v = (x @ Wv + bv).reshape(b, s, h, hd).transpose(0, 2, 1, 3)
    scores = np.einsum("bhqd,bhkd->bhqk", q, k) * (hd ** -0.5)
    scores = np.where(mask[:, None, None, :] == 0, -np.inf, scores)
    scores -= scores.max(axis=-1, keepdims=True)
    e = np.exp(scores)
    attn = e / e.sum(axis=-1, keepdims=True)
    ctx = np.einsum("bhqk,bhkd->bhqd", attn, v)
    ctx = ctx.transpose(0, 2, 1, 3).reshape(b, s, d)
    return (ctx @ Wo + bo).astype(np.float32)


def kernel(x, mask, Wq, bq, Wk, bk, Wv, bv, Wo, bo):
    x = np.asarray(x, dtype=np.float32)
    mask = np.asarray(mask)
    Wq, bq = np.asarray(Wq, np.float32), np.asarray(bq, np.float32)
    Wk, bk = np.asarray(Wk, np.float32), np.asarray(bk, np.float32)
    Wv, bv = np.asarray(Wv, np.float32), np.asarray(bv, np.float32)
    Wo, bo = np.asarray(Wo, np.float32), np.asarray(bo, np.float32)

    general = (
        x.shape != (B, S, D)
        or not np.all(mask == 1)
        or any(np.any(t != 0) for t in (bq, bk, bv))
    )
    if general:
        return _reference_numpy(x, mask, Wq, bq, Wk, bk, Wv, bv, Wo, bo)

    out, _ = _run_device(x, Wq, Wk, Wv, Wo, bo)
    return out
